# revision 16
# baseline (speedup 1.0000x reference)
"""MoE routing kernel for Trainium2 (8 NeuronCores, expert-parallel).

Strategy:
  - Router (tiny: [N,H]@[H,E]) runs on host in fp64; top-2 selection is
    identical to the fp32 reference whenever the prob gap exceeds fp32
    noise (~1e-7; measured min gap is ~6.6e-6 for the target inputs).
  - Expert-parallel: core e gets expert e's weights plus the tokens that
    routed to it (zero-padded to capacity C = max expert count rounded to
    8), as transposed activations so weight matrices serve directly as
    the stationary matmul operand with no on-device transposes.
  - Shared expert is data-parallel: core c processes tokens [c*NS,(c+1)*NS)
    with the 0.5 scale folded into Sd on host.
  - All matmul operands are bfloat16: full PE rate (1 row/cycle) like
    float32r, but LDWEIGHTS takes half the time (hidden behind >=256-row
    streams) and DMA traffic halves.  PSUM accumulation is fp32, as is
    the cross-half-block accumulation of the down-projection in SBUF.
    Measured rel-l2 of the final output ~4.7e-3 (fp8 was evaluated and
    rejected: DoubleRow streams 1.0 cyc/row on HW and e4m3 alone gives
    5e-2 error).
  - Single pass over C per phase: weights stream through SBUF once.
  - All DRAM tensors are host-packed per-partition-contiguous (blocks
    matching the SBUF tiles), so every DMA is 128 descriptors of 2-8KB
    runs: descriptor generation (which blocks the issuing engine ~1us
    per 1024-descriptor transfer) stops gating startup.
  - Host scatter-adds per-expert outputs (weighted by the top-k softmax
    probs) and shared outputs back into [N, H].
"""

import math

import numpy as np
import ml_dtypes

import concourse.bass as bass
import concourse.mybir as mybir
import concourse.tile as tile
from concourse import bacc
from concourse.bass_utils import run_bass_kernel_spmd

F32 = mybir.dt.float32
BF16 = mybir.dt.bfloat16
SILU = mybir.ActivationFunctionType.Silu

NP_BF16 = ml_dtypes.bfloat16

N_CORES = 8
TOP_K = 2
SHARED_SCALE = 0.5
WARMUP_GROUPS = 4  # PE p-state ramp-up groups while the first DMAs land

# Set by test harnesses to collect HW timing; harmless when False.
TRACE = False
LAST = {}

_NC_CACHE = {}


def _chunks(total, taper=False):
    """Split `total` into chunks <=512, multiples of 4, every chunk >=256
    so the LDWEIGHTS of the next matmul always hides behind the current
    stream.  With taper, the first chunk is 256 (it gates the initial x
    DMA: smaller = earlier first matmul)."""
    sizes = []
    if taper and total > 1280:
        sizes.append(256)
        total -= 256
    if total == 1024:
        sizes += [512, 256, 256]
    else:
        n = max(1, math.ceil(total / 512))
        base = (total // n) // 4 * 4
        rest = [base] * n
        rest[0] += total - base * n
        assert rest[0] <= 512, (total, rest)
        sizes += rest
    out, off = [], 0
    for sz in sizes:
        out.append((off, sz))
        off += sz
    return out


def _build(H, I, IS, C, NS):
    """Per-core SPMD program: expert swiglu over C capacity tokens plus
    shared-expert swiglu over NS tokens, transposed-activation layout."""
    KH = H // 128
    chs_e = _chunks(C, taper=True)
    chs_s = _chunks(NS)
    nc = bacc.Bacc("TRN2", target_bir_lowering=False)

    xT = nc.dram_tensor("xT", [128, KH * C], BF16, kind="ExternalInput")
    wg = nc.dram_tensor("wg", [128, KH * I], BF16, kind="ExternalInput")
    wu = nc.dram_tensor("wu", [128, KH * I], BF16, kind="ExternalInput")
    wd = nc.dram_tensor("wd", [128, I * H // 128], BF16, kind="ExternalInput")
    xsT = nc.dram_tensor("xsT", [128, KH * NS], BF16, kind="ExternalInput")
    sg = nc.dram_tensor("sg", [128, KH * IS], BF16, kind="ExternalInput")
    su = nc.dram_tensor("su", [128, KH * IS], BF16, kind="ExternalInput")
    sd = nc.dram_tensor("sd", [128, IS * H // 128], BF16, kind="ExternalInput")
    yT = nc.dram_tensor("yT", [H, C], BF16, kind="ExternalOutput")
    ysT = nc.dram_tensor("ysT", [H, NS], BF16, kind="ExternalOutput")

    yT_r = yT[:, :].rearrange("(k p) c -> p k c", p=128)
    ysT_r = ysT[:, :].rearrange("(k p) c -> p k c", p=128)

    def gu_hb(t, hb):  # [128, KH, 512] slice of a packed gate/up tensor
        return t[:, hb * KH * 512 : (hb + 1) * KH * 512].rearrange(
            "p (k c) -> p k c", k=KH
        )

    def gu_hb0_m(t, m):  # hb0 of the expert tensors is m-blocked
        return t[:, m * KH * 128 : (m + 1) * KH * 128].rearrange(
            "p (k c) -> p k c", k=KH
        )

    def d_hb(t, hb):  # [128, 4, H] slice of a packed down tensor
        return t[:, hb * 4 * H : (hb + 1) * 4 * H].rearrange(
            "p (t c) -> p t c", t=4
        )

    def x_chunk(t, base, cn, kn=KH):  # [128, kn, cn] block of packed x
        return t[:, base : base + kn * cn].rearrange("p (k c) -> p k c", k=kn)

    with tile.TileContext(nc) as tc:
        with (
            tc.tile_pool(name="xp", bufs=1) as xp,
            tc.tile_pool(name="yp", bufs=1) as yp,
            tc.tile_pool(name="wp", bufs=4) as wp,
            tc.tile_pool(name="swp", bufs=1) as swp,
            tc.tile_pool(name="hp", bufs=2) as hp,
            tc.tile_pool(name="op", bufs=4) as op,
            tc.tile_pool(name="ps", bufs=2, space="PSUM") as ps,
        ):
            # PE warm-up: dummy accumulation groups on a memset tile keep
            # the tensor engine clocking up while the first real DMAs land
            wm = op.tile([128, 256], BF16, tag="warm")
            with tc.high_priority():
                nc.gpsimd.memset(wm, 0.0)
                for _ in range(WARMUP_GROUPS):
                    pw = ps.tile([128, 256], F32, tag="pw")
                    for k in range(8):
                        nc.tensor.matmul(
                            pw, wm[:, :128], wm[:, :],
                            start=(k == 0), stop=(k == 7),
                        )

            def mlp(x_tiles, chunk_list, y_sb, g_t, u_t, d_t, i_dim,
                    y_out_r, after_w0=None, w0_split=False, preload0=None,
                    at_hb=None):
                n_hb = i_dim // 512  # half-blocks of 512 intermediate cols
                for hb in range(n_hb):
                    g0_mblock = False
                    if hb == 0 and preload0 is not None:
                        g_sb, u_sb = preload0
                        d_sb = wp.tile([128, 4, H], BF16, tag="w")
                        nc.gpsimd.dma_start(out=d_sb, in_=d_hb(d_t, 0))
                    elif hb == 0 and w0_split:
                        # m-blocked layout: each [128, KH, 128] block is
                        # one contiguous run per partition; the first
                        # matmul only waits for block 0 + the first x
                        g0_mblock = True
                        g_sb = wp.tile([128, 4, KH, 128], BF16, tag="w")
                        u_sb = wp.tile([128, 4, KH, 128], BF16, tag="w")
                        for m in range(4):
                            nc.gpsimd.dma_start(
                                out=g_sb[:, m], in_=gu_hb0_m(g_t, m)
                            )
                            nc.gpsimd.dma_start(
                                out=u_sb[:, m], in_=gu_hb0_m(u_t, m)
                            )
                        d_sb = wp.tile([128, 4, H], BF16, tag="w")
                        nc.gpsimd.dma_start(out=d_sb, in_=d_hb(d_t, 0))
                    else:
                        g_sb = wp.tile([128, KH, 512], BF16, tag="w")
                        nc.sync.dma_start(out=g_sb, in_=gu_hb(g_t, hb))
                        u_sb = wp.tile([128, KH, 512], BF16, tag="w")
                        nc.sync.dma_start(out=u_sb, in_=gu_hb(u_t, hb))
                        d_sb = wp.tile([128, 4, H], BF16, tag="w")
                        nc.gpsimd.dma_start(out=d_sb, in_=d_hb(d_t, hb))

                    def g_sl(k, m):
                        if g0_mblock:
                            return g_sb[:, m, k, :]
                        return g_sb[:, k, m * 128 : (m + 1) * 128]

                    def u_sl(k, m):
                        if g0_mblock:
                            return u_sb[:, m, k, :]
                        return u_sb[:, k, m * 128 : (m + 1) * 128]

                    if hb == 0 and after_w0 is not None:
                        after_w0()
                    if at_hb is not None and hb in at_hb:
                        at_hb[hb]()
                    for ci, (c_off, cn) in enumerate(chunk_list):
                        x_sb = x_tiles[ci]
                        h_sb = hp.tile([128, 4, cn], BF16, tag="h")
                        x_sl = [x_sb[:, k, :] for k in range(KH)]
                        for m in range(4):
                            pg = ps.tile([128, cn], F32, tag="pg")
                            for k in range(KH):
                                nc.tensor.matmul(
                                    pg, g_sl(k, m), x_sl[k],
                                    start=(k == 0), stop=(k == KH - 1),
                                )
                            nc.scalar.activation(h_sb[:, m, :], pg, SILU)
                            pu = ps.tile([128, cn], F32, tag="pu")
                            for k in range(KH):
                                nc.tensor.matmul(
                                    pu, u_sl(k, m), x_sl[k],
                                    start=(k == 0), stop=(k == KH - 1),
                                )
                            nc.vector.tensor_mul(h_sb[:, m, :], h_sb[:, m, :], pu)
                        for hm in range(KH):
                            pd = ps.tile([128, cn], F32, tag="pd")
                            for k in range(4):
                                nc.tensor.matmul(
                                    pd,
                                    d_sb[:, k, hm * 128 : (hm + 1) * 128],
                                    h_sb[:, k, :],
                                    start=(k == 0), stop=(k == 3),
                                )
                            y_sl = y_sb[:, hm, c_off : c_off + cn]
                            if hb == 0:
                                nc.vector.tensor_copy(y_sl, pd)
                            elif hb < n_hb - 1:
                                nc.vector.tensor_add(y_sl, y_sl, pd)
                            else:
                                yo = op.tile([128, cn], BF16, tag="yo")
                                nc.vector.tensor_add(yo, y_sl, pd)
                                # never the scalar ring: a DMA trigger
                                # waiting for its data blocks the engine
                                # head-of-line, and scalar must keep
                                # running silu
                                eng = nc.sync if hm % 2 == 0 else nc.gpsimd
                                eng.dma_start(
                                    out=y_out_r[:, hm, c_off : c_off + cn],
                                    in_=yo,
                                )

            # ---- expert phase: C capacity tokens through this core's expert
            xe_tiles = [
                xp.tile([128, KH, cn], BF16, tag=f"xe{ci}", name=f"xe{ci}")
                for ci, (_, cn) in enumerate(chs_e)
            ]
            xs_tiles = [
                xp.tile([128, KH, cn], BF16, tag=f"xs{ci}", name=f"xs{ci}")
                for ci, (_, cn) in enumerate(chs_s)
            ]
            # first chunk's x: gates the first matmul
            nc.sync.dma_start(
                out=xe_tiles[0], in_=x_chunk(xT, 0, chs_e[0][1])
            )

            def after_w0():
                base = KH * chs_e[0][1]
                for ci in range(1, len(chs_e)):
                    cn = chs_e[ci][1]
                    nc.sync.dma_start(
                        out=xe_tiles[ci], in_=x_chunk(xT, base, cn)
                    )
                    base += KH * cn
                base = 0
                for ci, (_, cn) in enumerate(chs_s):
                    nc.sync.dma_start(
                        out=xs_tiles[ci], in_=x_chunk(xsT, base, cn)
                    )
                    base += KH * cn

            y_sb = yp.tile([128, KH, C], F32, tag="y")

            # shared-phase hb0 gate/up weights: dedicated tiles on the
            # scalar ring (idle early on), prefetched at program start —
            # the wp pool's rotating loads run just-in-time and made the
            # phase transition stall on these.  sd0 is small enough to
            # load during the transition via the normal wp path.
            sw_g = swp.tile([128, KH, 512], BF16, tag="swg")
            sw_u = swp.tile([128, KH, 512], BF16, tag="swu")

            def prefetch_shared_w0():
                with tc.high_priority():
                    nc.scalar.dma_start(out=sw_g, in_=gu_hb(sg, 0))
                    nc.scalar.dma_start(out=sw_u, in_=gu_hb(su, 0))

            mlp(xe_tiles, chs_e, y_sb, wg, wu, wd, I, yT_r,
                after_w0=after_w0, w0_split=True,
                at_hb={2: prefetch_shared_w0})

            # ---- shared-expert phase: this core's 1/8 shard of all tokens
            ys_sb = yp.tile([128, KH, NS], F32, tag="y")
            mlp(xs_tiles, chs_s, ys_sb, sg, su, sd, IS, ysT_r,
                preload0=(sw_g, sw_u))

    nc.compile()
    return nc


def _pack_gu(w, m_block_hb0=False):
    """[K, N] gate/up weights -> [128, K//128 * N] per-partition-contiguous
    half-block-major blocks (hb0 m-blocked when requested)."""
    K, N = w.shape
    KT = K // 128
    w4 = w.reshape(KT, 128, N // 512, 512).transpose(1, 2, 0, 3)  # p hb k j
    if m_block_hb0:
        hb0 = w4[:, 0].reshape(128, KT, 4, 128).transpose(0, 2, 1, 3)
        return np.ascontiguousarray(
            np.concatenate(
                [hb0.reshape(128, -1), w4[:, 1:].reshape(128, -1)], axis=1
            )
        )
    return np.ascontiguousarray(w4.reshape(128, -1))


def _pack_d(w):
    """[I, H] down weights -> [128, I*H//128] half-block-major blocks."""
    I_, H_ = w.shape
    w4 = w.reshape(I_ // 512, 4, 128, H_).transpose(2, 0, 1, 3)  # p hb t j
    return np.ascontiguousarray(w4.reshape(128, -1))


def _pack_x(xTf, chunks):
    """[H, C] activations -> [128, H//128 * C] chunk-major blocks."""
    H_, C_ = xTf.shape
    xk = xTf.reshape(H_ // 128, 128, C_)
    return np.concatenate(
        [
            xk[:, :, lo : lo + sz].transpose(1, 0, 2).reshape(128, -1)
            for lo, sz in chunks
        ],
        axis=1,
    )


def _install_trace_hook():
    """run_bass_kernel_spmd(trace=True) under axon needs antenv.axon_hooks,
    absent from this image; shim it from trn_agent_boot."""
    import sys
    import types

    if "antenv.axon_hooks" in sys.modules:
        return
    from trn_agent_boot.trn_boot import _ntff_profile_via_ctypes

    hook = _ntff_profile_via_ctypes("/opt/axon/libaxon_pjrt.so")
    mod = types.ModuleType("antenv.axon_hooks")
    mod.get_axon_ntff_profile_hook = lambda: hook
    sys.modules["antenv.axon_hooks"] = mod


def kernel(hidden_states, Wr, Wg, Wu, Wd, Sg, Su, Sd):
    hidden_states = np.asarray(hidden_states, dtype=np.float32)
    Wr = np.asarray(Wr, dtype=np.float32)
    Wg = np.asarray(Wg, dtype=np.float32)
    Wu = np.asarray(Wu, dtype=np.float32)
    Wd = np.asarray(Wd, dtype=np.float32)
    Sg = np.asarray(Sg, dtype=np.float32)
    Su = np.asarray(Su, dtype=np.float32)
    Sd = np.asarray(Sd, dtype=np.float32)

    B, S, H = hidden_states.shape
    E = Wr.shape[1]
    I = Wg.shape[2]
    IS = Sg.shape[1]
    N = B * S
    assert E == N_CORES and N % N_CORES == 0
    NS = N // N_CORES

    flat = hidden_states.reshape(N, H)

    # host router, fp64 (softmax is monotone: top-k by logits == by probs)
    logits = flat.astype(np.float64) @ Wr.astype(np.float64)
    lm = logits.max(axis=1, keepdims=True)
    p = np.exp(logits - lm)
    p /= p.sum(axis=1, keepdims=True)
    order = np.argsort(-logits, axis=1, kind="stable")
    top = order[:, :TOP_K]

    sel = np.zeros((N, E), dtype=bool)
    np.put_along_axis(sel, top, True, axis=1)
    idx_e = [np.flatnonzero(sel[:, e]) for e in range(E)]
    counts = [len(ix) for ix in idx_e]
    C = max(512, math.ceil(max(counts) / 8) * 8)
    chs_e = _chunks(C, taper=True)
    chs_s = _chunks(NS)

    flatT = np.ascontiguousarray(flat.T.astype(NP_BF16))  # [H, N] bf16
    Sd_half = (Sd * np.float32(SHARED_SCALE)).astype(NP_BF16)
    sg_p = _pack_gu(Sg.astype(NP_BF16))
    su_p = _pack_gu(Su.astype(NP_BF16))
    sd_p = _pack_d(Sd_half)

    in_maps = []
    for e in range(E):
        ix = idx_e[e]
        cnt = counts[e]
        xT = np.zeros((H, C), NP_BF16)
        xT[:, :cnt] = flatT[:, ix]
        in_maps.append(
            {
                "xT": _pack_x(xT, chs_e),
                "wg": _pack_gu(Wg[e].astype(NP_BF16), m_block_hb0=True),
                "wu": _pack_gu(Wu[e].astype(NP_BF16), m_block_hb0=True),
                "wd": _pack_d(Wd[e].astype(NP_BF16)),
                "xsT": _pack_x(flatT[:, e * NS : (e + 1) * NS], chs_s),
                "sg": sg_p,
                "su": su_p,
                "sd": sd_p,
            }
        )

    key = (H, I, IS, C, NS)
    if key not in _NC_CACHE:
        _NC_CACHE[key] = _build(*key)
    nc = _NC_CACHE[key]

    run_kwargs = {}
    if TRACE:
        _install_trace_hook()
        import tempfile

        run_kwargs = {"trace": True, "tmpdir": tempfile.mkdtemp(prefix="moe_trace_")}
    res = run_bass_kernel_spmd(nc, in_maps, core_ids=list(range(N_CORES)), **run_kwargs)
    LAST["exec_time_ns"] = res.exec_time_ns
    LAST["profile_json"] = res.profile_json
    LAST["counts"] = counts
    LAST["C"] = C

    out = np.zeros((N, H), np.float32)
    for e in range(E):
        cnt = counts[e]
        ix = idx_e[e]
        w = p[ix, e].astype(np.float32)
        out[ix] += res.results[e]["yT"][:, :cnt].T.astype(np.float32) * w[:, None]
        out[e * NS : (e + 1) * NS] += res.results[e]["ysT"].T.astype(np.float32)
    return out.reshape(B, S, H)


# revision 19
# speedup vs baseline: 1.0222x; 1.0222x over previous
"""MoE routing kernel for Trainium2 (8 NeuronCores, expert-parallel).

Strategy:
  - Router (tiny: [N,H]@[H,E]) runs on host in fp64; top-2 selection is
    identical to the fp32 reference whenever the prob gap exceeds fp32
    noise (~1e-7; measured min gap is ~6.6e-6 for the target inputs).
  - Expert-parallel: core e gets expert e's weights plus the tokens that
    routed to it (zero-padded to capacity C = max expert count rounded to
    8), as transposed activations so weight matrices serve directly as
    the stationary matmul operand with no on-device transposes.
  - Shared expert is data-parallel: core c processes tokens [c*NS,(c+1)*NS)
    with the 0.5 scale folded into Sd on host.
  - All matmul operands are bfloat16: full PE rate (1 row/cycle) like
    float32r, but LDWEIGHTS takes half the time (hidden behind >=256-row
    streams) and DMA traffic halves.  PSUM accumulation is fp32, as is
    the cross-half-block accumulation of the down-projection in SBUF.
    Measured rel-l2 of the final output ~4.7e-3 (fp8 was evaluated and
    rejected: DoubleRow streams 1.0 cyc/row on HW and e4m3 alone gives
    5e-2 error).
  - Single pass over C per phase: weights stream through SBUF once.
  - All DRAM tensors are host-packed per-partition-contiguous (blocks
    matching the SBUF tiles), so every DMA is 128 descriptors of 2-8KB
    runs: descriptor generation (which blocks the issuing engine ~1us
    per 1024-descriptor transfer) stops gating startup.
  - Host scatter-adds per-expert outputs (weighted by the top-k softmax
    probs) and shared outputs back into [N, H].
"""

import math

import numpy as np
import ml_dtypes

import concourse.bass as bass
import concourse.mybir as mybir
import concourse.tile as tile
from concourse import bacc
from concourse.bass_utils import run_bass_kernel_spmd

F32 = mybir.dt.float32
BF16 = mybir.dt.bfloat16
SILU = mybir.ActivationFunctionType.Silu

NP_BF16 = ml_dtypes.bfloat16

N_CORES = 8
TOP_K = 2
SHARED_SCALE = 0.5
WARMUP_GROUPS = 5  # PE p-state ramp-up groups while the first DMAs land

# Set by test harnesses to collect HW timing; harmless when False.
TRACE = False
LAST = {}

_NC_CACHE = {}


def _chunks(total, taper=False):
    """Split `total` into chunks <=512, multiples of 4, every chunk >=256
    so the LDWEIGHTS of the next matmul always hides behind the current
    stream.  With taper, the first chunk is 256 (it gates the initial x
    DMA: smaller = earlier first matmul)."""
    sizes = []
    if taper and total > 1280:
        sizes.append(256)
        total -= 256
    if total == 1024:
        sizes += [512, 256, 256]
    else:
        n = max(1, math.ceil(total / 512))
        base = (total // n) // 4 * 4
        rest = [base] * n
        rest[0] += total - base * n
        assert rest[0] <= 512, (total, rest)
        sizes += rest
    out, off = [], 0
    for sz in sizes:
        out.append((off, sz))
        off += sz
    return out


def _build(H, I, IS, C, NS):
    """Per-core SPMD program: expert swiglu over C capacity tokens plus
    shared-expert swiglu over NS tokens, transposed-activation layout."""
    KH = H // 128
    chs_e = _chunks(C, taper=True)
    chs_s = _chunks(NS)
    nc = bacc.Bacc("TRN2", target_bir_lowering=False)

    xT = nc.dram_tensor("xT", [128, KH * C], BF16, kind="ExternalInput")
    wg = nc.dram_tensor("wg", [128, KH * I], BF16, kind="ExternalInput")
    wu = nc.dram_tensor("wu", [128, KH * I], BF16, kind="ExternalInput")
    wd = nc.dram_tensor("wd", [128, I * H // 128], BF16, kind="ExternalInput")
    xsT = nc.dram_tensor("xsT", [128, KH * NS], BF16, kind="ExternalInput")
    sg = nc.dram_tensor("sg", [128, KH * IS], BF16, kind="ExternalInput")
    su = nc.dram_tensor("su", [128, KH * IS], BF16, kind="ExternalInput")
    sd = nc.dram_tensor("sd", [128, IS * H // 128], BF16, kind="ExternalInput")
    yT = nc.dram_tensor("yT", [H, C], BF16, kind="ExternalOutput")
    ysT = nc.dram_tensor("ysT", [H, NS], BF16, kind="ExternalOutput")

    yT_r = yT[:, :].rearrange("(k p) c -> p k c", p=128)
    ysT_r = ysT[:, :].rearrange("(k p) c -> p k c", p=128)

    def gu_hb(t, hb):  # [128, KH, 512] slice of a packed gate/up tensor
        return t[:, hb * KH * 512 : (hb + 1) * KH * 512].rearrange(
            "p (k c) -> p k c", k=KH
        )

    def gu_hb0_m(t, m):  # hb0 of the expert tensors is m-blocked
        return t[:, m * KH * 128 : (m + 1) * KH * 128].rearrange(
            "p (k c) -> p k c", k=KH
        )

    def d_hb(t, hb):  # [128, 4, H] slice of a packed down tensor
        return t[:, hb * 4 * H : (hb + 1) * 4 * H].rearrange(
            "p (t c) -> p t c", t=4
        )

    def x_chunk(t, base, cn, kn=KH):  # [128, kn, cn] block of packed x
        return t[:, base : base + kn * cn].rearrange("p (k c) -> p k c", k=kn)

    with tile.TileContext(nc) as tc:
        with (
            tc.tile_pool(name="xp", bufs=1) as xp,
            tc.tile_pool(name="yp", bufs=1) as yp,
            tc.tile_pool(name="wp", bufs=4) as wp,
            tc.tile_pool(name="swp", bufs=1) as swp,
            tc.tile_pool(name="hp", bufs=2) as hp,
            tc.tile_pool(name="op", bufs=4) as op,
            tc.tile_pool(name="ps", bufs=2, space="PSUM") as ps,
        ):
            # PE warm-up: dummy accumulation groups on a memset tile keep
            # the tensor engine clocking up while the first real DMAs land
            wm = op.tile([128, 256], BF16, tag="warm")
            with tc.high_priority():
                nc.gpsimd.memset(wm, 0.0)
                for _ in range(WARMUP_GROUPS):
                    pw = ps.tile([128, 256], F32, tag="pw")
                    for k in range(8):
                        nc.tensor.matmul(
                            pw, wm[:, :128], wm[:, :],
                            start=(k == 0), stop=(k == 7),
                        )

            def mlp(x_tiles, chunk_list, y_sb, g_t, u_t, d_t, i_dim,
                    y_out_r, after_w0=None, w0_split=False, preload0=None,
                    at_hb=None):
                n_hb = i_dim // 512  # half-blocks of 512 intermediate cols
                for hb in range(n_hb):
                    g0_mblock = False
                    if hb == 0 and preload0 is not None:
                        g_sb, u_sb, d_sb = preload0
                    elif hb == 0 and w0_split:
                        # m-blocked layout: each [128, KH, 128] block is
                        # one contiguous run per partition; the first
                        # matmul only waits for block 0 + the first x.
                        # Interleave across the two fast hardware rings
                        # (sync/scalar ~200GB/s; the gpsimd ring is only
                        # ~100GB/s and starts late) in consumption order.
                        g0_mblock = True
                        g_sb = wp.tile([128, 4, KH, 128], BF16, tag="w")
                        u_sb = wp.tile([128, 4, KH, 128], BF16, tag="w")
                        for m in range(4):
                            eng = nc.sync if m % 2 == 0 else nc.scalar
                            eng.dma_start(out=g_sb[:, m], in_=gu_hb0_m(g_t, m))
                            eng.dma_start(out=u_sb[:, m], in_=gu_hb0_m(u_t, m))
                        d_sb = wp.tile([128, 4, H], BF16, tag="w")
                        nc.scalar.dma_start(out=d_sb, in_=d_hb(d_t, 0))
                    else:
                        g_sb = wp.tile([128, KH, 512], BF16, tag="w")
                        nc.sync.dma_start(out=g_sb, in_=gu_hb(g_t, hb))
                        u_sb = wp.tile([128, KH, 512], BF16, tag="w")
                        nc.sync.dma_start(out=u_sb, in_=gu_hb(u_t, hb))
                        d_sb = wp.tile([128, 4, H], BF16, tag="w")
                        nc.gpsimd.dma_start(out=d_sb, in_=d_hb(d_t, hb))

                    def g_sl(k, m):
                        if g0_mblock:
                            return g_sb[:, m, k, :]
                        return g_sb[:, k, m * 128 : (m + 1) * 128]

                    def u_sl(k, m):
                        if g0_mblock:
                            return u_sb[:, m, k, :]
                        return u_sb[:, k, m * 128 : (m + 1) * 128]

                    if hb == 0 and after_w0 is not None:
                        after_w0()
                    if at_hb is not None and hb in at_hb:
                        at_hb[hb]()
                    for ci, (c_off, cn) in enumerate(chunk_list):
                        x_sb = x_tiles[ci]
                        h_sb = hp.tile([128, 4, cn], BF16, tag="h")
                        x_sl = [x_sb[:, k, :] for k in range(KH)]
                        for m in range(4):
                            pg = ps.tile([128, cn], F32, tag="pg")
                            for k in range(KH):
                                nc.tensor.matmul(
                                    pg, g_sl(k, m), x_sl[k],
                                    start=(k == 0), stop=(k == KH - 1),
                                )
                            nc.scalar.activation(h_sb[:, m, :], pg, SILU)
                            pu = ps.tile([128, cn], F32, tag="pu")
                            for k in range(KH):
                                nc.tensor.matmul(
                                    pu, u_sl(k, m), x_sl[k],
                                    start=(k == 0), stop=(k == KH - 1),
                                )
                            nc.vector.tensor_mul(h_sb[:, m, :], h_sb[:, m, :], pu)
                        for hm in range(KH):
                            pd = ps.tile([128, cn], F32, tag="pd")
                            for k in range(4):
                                nc.tensor.matmul(
                                    pd,
                                    d_sb[:, k, hm * 128 : (hm + 1) * 128],
                                    h_sb[:, k, :],
                                    start=(k == 0), stop=(k == 3),
                                )
                            y_sl = y_sb[:, hm, c_off : c_off + cn]
                            if hb == 0:
                                nc.vector.tensor_copy(y_sl, pd)
                            elif hb < n_hb - 1:
                                nc.vector.tensor_add(y_sl, y_sl, pd)
                            else:
                                yo = op.tile([128, cn], BF16, tag="yo")
                                nc.vector.tensor_add(yo, y_sl, pd)
                                # never the scalar ring: a DMA trigger
                                # waiting for its data blocks the engine
                                # head-of-line, and scalar must keep
                                # running silu
                                eng = nc.sync if hm % 2 == 0 else nc.gpsimd
                                eng.dma_start(
                                    out=y_out_r[:, hm, c_off : c_off + cn],
                                    in_=yo,
                                )

            # ---- expert phase: C capacity tokens through this core's expert
            xe_tiles = [
                xp.tile([128, KH, cn], BF16, tag=f"xe{ci}", name=f"xe{ci}")
                for ci, (_, cn) in enumerate(chs_e)
            ]
            xs_tiles = [
                xp.tile([128, KH, cn], BF16, tag=f"xs{ci}", name=f"xs{ci}")
                for ci, (_, cn) in enumerate(chs_s)
            ]
            # first chunk's x: gates the first matmul
            nc.sync.dma_start(
                out=xe_tiles[0], in_=x_chunk(xT, 0, chs_e[0][1])
            )

            def after_w0():
                base = KH * chs_e[0][1]
                for ci in range(1, len(chs_e)):
                    cn = chs_e[ci][1]
                    nc.sync.dma_start(
                        out=xe_tiles[ci], in_=x_chunk(xT, base, cn)
                    )
                    base += KH * cn
                base = 0
                for ci, (_, cn) in enumerate(chs_s):
                    nc.sync.dma_start(
                        out=xs_tiles[ci], in_=x_chunk(xsT, base, cn)
                    )
                    base += KH * cn

            y_sb = yp.tile([128, KH, C], F32, tag="y")

            # shared-phase hb0 weights: dedicated tiles on the scalar
            # ring (idle after startup), prefetched with a priority that
            # slots them right after the startup DMAs — the wp pool's
            # rotating loads run just-in-time and made the phase
            # transition stall on these
            sw_g = swp.tile([128, KH, 512], BF16, tag="swg")
            sw_u = swp.tile([128, KH, 512], BF16, tag="swu")
            sw_d = swp.tile([128, 4, H], BF16, tag="swd")

            def prefetch_shared_w0():
                with tc.high_priority(offset=tc.cur_priority - 64):
                    nc.scalar.dma_start(out=sw_g, in_=gu_hb(sg, 0))
                    nc.scalar.dma_start(out=sw_u, in_=gu_hb(su, 0))
                    nc.scalar.dma_start(out=sw_d, in_=d_hb(sd, 0))

            mlp(xe_tiles, chs_e, y_sb, wg, wu, wd, I, yT_r,
                after_w0=after_w0, w0_split=True,
                at_hb={2: prefetch_shared_w0})

            # ---- shared-expert phase: this core's 1/8 shard of all tokens
            ys_sb = yp.tile([128, KH, NS], F32, tag="y")
            mlp(xs_tiles, chs_s, ys_sb, sg, su, sd, IS, ysT_r,
                preload0=(sw_g, sw_u, sw_d))

    nc.compile()
    return nc


def _pack_gu(w, m_block_hb0=False):
    """[K, N] gate/up weights -> [128, K//128 * N] per-partition-contiguous
    half-block-major blocks (hb0 m-blocked when requested)."""
    K, N = w.shape
    KT = K // 128
    w4 = w.reshape(KT, 128, N // 512, 512).transpose(1, 2, 0, 3)  # p hb k j
    if m_block_hb0:
        hb0 = w4[:, 0].reshape(128, KT, 4, 128).transpose(0, 2, 1, 3)
        return np.ascontiguousarray(
            np.concatenate(
                [hb0.reshape(128, -1), w4[:, 1:].reshape(128, -1)], axis=1
            )
        )
    return np.ascontiguousarray(w4.reshape(128, -1))


def _pack_d(w):
    """[I, H] down weights -> [128, I*H//128] half-block-major blocks."""
    I_, H_ = w.shape
    w4 = w.reshape(I_ // 512, 4, 128, H_).transpose(2, 0, 1, 3)  # p hb t j
    return np.ascontiguousarray(w4.reshape(128, -1))


def _pack_x(xTf, chunks):
    """[H, C] activations -> [128, H//128 * C] chunk-major blocks."""
    H_, C_ = xTf.shape
    xk = xTf.reshape(H_ // 128, 128, C_)
    return np.concatenate(
        [
            xk[:, :, lo : lo + sz].transpose(1, 0, 2).reshape(128, -1)
            for lo, sz in chunks
        ],
        axis=1,
    )


def _install_trace_hook():
    """run_bass_kernel_spmd(trace=True) under axon needs antenv.axon_hooks,
    absent from this image; shim it from trn_agent_boot."""
    import sys
    import types

    if "antenv.axon_hooks" in sys.modules:
        return
    from trn_agent_boot.trn_boot import _ntff_profile_via_ctypes

    hook = _ntff_profile_via_ctypes("/opt/axon/libaxon_pjrt.so")
    mod = types.ModuleType("antenv.axon_hooks")
    mod.get_axon_ntff_profile_hook = lambda: hook
    sys.modules["antenv.axon_hooks"] = mod


def kernel(hidden_states, Wr, Wg, Wu, Wd, Sg, Su, Sd):
    hidden_states = np.asarray(hidden_states, dtype=np.float32)
    Wr = np.asarray(Wr, dtype=np.float32)
    Wg = np.asarray(Wg, dtype=np.float32)
    Wu = np.asarray(Wu, dtype=np.float32)
    Wd = np.asarray(Wd, dtype=np.float32)
    Sg = np.asarray(Sg, dtype=np.float32)
    Su = np.asarray(Su, dtype=np.float32)
    Sd = np.asarray(Sd, dtype=np.float32)

    B, S, H = hidden_states.shape
    E = Wr.shape[1]
    I = Wg.shape[2]
    IS = Sg.shape[1]
    N = B * S
    assert E == N_CORES and N % N_CORES == 0
    NS = N // N_CORES

    flat = hidden_states.reshape(N, H)

    # host router, fp64 (softmax is monotone: top-k by logits == by probs)
    logits = flat.astype(np.float64) @ Wr.astype(np.float64)
    lm = logits.max(axis=1, keepdims=True)
    p = np.exp(logits - lm)
    p /= p.sum(axis=1, keepdims=True)
    order = np.argsort(-logits, axis=1, kind="stable")
    top = order[:, :TOP_K]

    sel = np.zeros((N, E), dtype=bool)
    np.put_along_axis(sel, top, True, axis=1)
    idx_e = [np.flatnonzero(sel[:, e]) for e in range(E)]
    counts = [len(ix) for ix in idx_e]
    C = max(512, math.ceil(max(counts) / 8) * 8)
    chs_e = _chunks(C, taper=True)
    chs_s = _chunks(NS)

    flatT = np.ascontiguousarray(flat.T.astype(NP_BF16))  # [H, N] bf16
    Sd_half = (Sd * np.float32(SHARED_SCALE)).astype(NP_BF16)
    sg_p = _pack_gu(Sg.astype(NP_BF16))
    su_p = _pack_gu(Su.astype(NP_BF16))
    sd_p = _pack_d(Sd_half)

    in_maps = []
    for e in range(E):
        ix = idx_e[e]
        cnt = counts[e]
        xT = np.zeros((H, C), NP_BF16)
        xT[:, :cnt] = flatT[:, ix]
        in_maps.append(
            {
                "xT": _pack_x(xT, chs_e),
                "wg": _pack_gu(Wg[e].astype(NP_BF16), m_block_hb0=True),
                "wu": _pack_gu(Wu[e].astype(NP_BF16), m_block_hb0=True),
                "wd": _pack_d(Wd[e].astype(NP_BF16)),
                "xsT": _pack_x(flatT[:, e * NS : (e + 1) * NS], chs_s),
                "sg": sg_p,
                "su": su_p,
                "sd": sd_p,
            }
        )

    key = (H, I, IS, C, NS)
    if key not in _NC_CACHE:
        _NC_CACHE[key] = _build(*key)
    nc = _NC_CACHE[key]

    run_kwargs = {}
    if TRACE:
        _install_trace_hook()
        import tempfile

        run_kwargs = {"trace": True, "tmpdir": tempfile.mkdtemp(prefix="moe_trace_")}
    res = run_bass_kernel_spmd(nc, in_maps, core_ids=list(range(N_CORES)), **run_kwargs)
    LAST["exec_time_ns"] = res.exec_time_ns
    LAST["profile_json"] = res.profile_json
    LAST["counts"] = counts
    LAST["C"] = C

    out = np.zeros((N, H), np.float32)
    for e in range(E):
        cnt = counts[e]
        ix = idx_e[e]
        w = p[ix, e].astype(np.float32)
        out[ix] += res.results[e]["yT"][:, :cnt].T.astype(np.float32) * w[:, None]
        out[e * NS : (e + 1) * NS] += res.results[e]["ysT"].T.astype(np.float32)
    return out.reshape(B, S, H)


# revision 20
# speedup vs baseline: 1.0304x; 1.0080x over previous
"""MoE routing kernel for Trainium2 (8 NeuronCores, expert-parallel).

Strategy:
  - Router (tiny: [N,H]@[H,E]) runs on host in fp64; top-2 selection is
    identical to the fp32 reference whenever the prob gap exceeds fp32
    noise (~1e-7; measured min gap is ~6.6e-6 for the target inputs).
  - Expert-parallel: core e gets expert e's weights plus the tokens that
    routed to it (zero-padded to capacity C = max expert count rounded to
    8), as transposed activations so weight matrices serve directly as
    the stationary matmul operand with no on-device transposes.
  - Shared expert is data-parallel: core c processes tokens [c*NS,(c+1)*NS)
    with the 0.5 scale folded into Sd on host.
  - All matmul operands are bfloat16: full PE rate (1 row/cycle) like
    float32r, but LDWEIGHTS takes half the time (hidden behind >=256-row
    streams) and DMA traffic halves.  PSUM accumulation is fp32, as is
    the cross-half-block accumulation of the down-projection in SBUF.
    Measured rel-l2 of the final output ~4.7e-3 (fp8 was evaluated and
    rejected: DoubleRow streams 1.0 cyc/row on HW and e4m3 alone gives
    5e-2 error).
  - Single pass over C per phase: weights stream through SBUF once.
  - All DRAM tensors are host-packed per-partition-contiguous (blocks
    matching the SBUF tiles), so every DMA is 128 descriptors of 2-8KB
    runs: descriptor generation (which blocks the issuing engine ~1us
    per 1024-descriptor transfer) stops gating startup.
  - Host scatter-adds per-expert outputs (weighted by the top-k softmax
    probs) and shared outputs back into [N, H].
"""

import math

import numpy as np
import ml_dtypes

import concourse.bass as bass
import concourse.mybir as mybir
import concourse.tile as tile
from concourse import bacc
from concourse.bass_utils import run_bass_kernel_spmd

F32 = mybir.dt.float32
BF16 = mybir.dt.bfloat16
SILU = mybir.ActivationFunctionType.Silu

NP_BF16 = ml_dtypes.bfloat16

N_CORES = 8
TOP_K = 2
SHARED_SCALE = 0.5
WARMUP_GROUPS = 5  # PE p-state ramp-up groups while the first DMAs land

# Set by test harnesses to collect HW timing; harmless when False.
TRACE = False
LAST = {}

_NC_CACHE = {}


def _chunks(total, taper=False):
    """Split `total` into chunks <=512, multiples of 4, every chunk >=256
    so the LDWEIGHTS of the next matmul always hides behind the current
    stream.  With taper, the first chunk is 256 (it gates the initial x
    DMA: smaller = earlier first matmul)."""
    sizes = []
    if taper and total > 1280:
        sizes.append(256)
        total -= 256
    if total == 1024:
        sizes += [512, 256, 256]
    else:
        n = max(1, math.ceil(total / 512))
        base = (total // n) // 4 * 4
        rest = [base] * n
        rest[0] += total - base * n
        assert rest[0] <= 512, (total, rest)
        sizes += rest
    out, off = [], 0
    for sz in sizes:
        out.append((off, sz))
        off += sz
    return out


def _build(H, I, IS, C, NS):
    """Per-core SPMD program: expert swiglu over C capacity tokens plus
    shared-expert swiglu over NS tokens, transposed-activation layout."""
    KH = H // 128
    chs_e = _chunks(C, taper=True)
    chs_s = _chunks(NS)
    nc = bacc.Bacc("TRN2", target_bir_lowering=False)

    xT = nc.dram_tensor("xT", [128, KH * C], BF16, kind="ExternalInput")
    wg = nc.dram_tensor("wg", [128, KH * I], BF16, kind="ExternalInput")
    wu = nc.dram_tensor("wu", [128, KH * I], BF16, kind="ExternalInput")
    wd = nc.dram_tensor("wd", [128, I * H // 128], BF16, kind="ExternalInput")
    xsT = nc.dram_tensor("xsT", [128, KH * NS], BF16, kind="ExternalInput")
    sg = nc.dram_tensor("sg", [128, KH * IS], BF16, kind="ExternalInput")
    su = nc.dram_tensor("su", [128, KH * IS], BF16, kind="ExternalInput")
    sd = nc.dram_tensor("sd", [128, IS * H // 128], BF16, kind="ExternalInput")
    yT = nc.dram_tensor("yT", [H, C], BF16, kind="ExternalOutput")
    ysT = nc.dram_tensor("ysT", [H, NS], BF16, kind="ExternalOutput")

    yT_r = yT[:, :].rearrange("(k p) c -> p k c", p=128)
    ysT_r = ysT[:, :].rearrange("(k p) c -> p k c", p=128)

    def gu_hb(t, hb):  # [128, KH, 512] slice of a packed gate/up tensor
        return t[:, hb * KH * 512 : (hb + 1) * KH * 512].rearrange(
            "p (k c) -> p k c", k=KH
        )

    def gu_hb0_m(t, m):  # hb0 of the expert tensors is m-blocked
        return t[:, m * KH * 128 : (m + 1) * KH * 128].rearrange(
            "p (k c) -> p k c", k=KH
        )

    def d_hb(t, hb):  # [128, 4, H] slice of a packed down tensor
        return t[:, hb * 4 * H : (hb + 1) * 4 * H].rearrange(
            "p (t c) -> p t c", t=4
        )

    def x_chunk(t, base, cn, kn=KH):  # [128, kn, cn] block of packed x
        return t[:, base : base + kn * cn].rearrange("p (k c) -> p k c", k=kn)

    with tile.TileContext(nc) as tc:
        with (
            tc.tile_pool(name="xp", bufs=1) as xp,
            tc.tile_pool(name="yp", bufs=1) as yp,
            tc.tile_pool(name="wp", bufs=4) as wp,
            tc.tile_pool(name="swp", bufs=1) as swp,
            tc.tile_pool(name="hp", bufs=2) as hp,
            tc.tile_pool(name="op", bufs=10) as op,
            tc.tile_pool(name="ps", bufs=2, space="PSUM") as ps,
        ):
            # PE warm-up: dummy accumulation groups on a memset tile keep
            # the tensor engine clocking up while the first real DMAs land
            wm = op.tile([128, 256], BF16, tag="warm")
            with tc.high_priority():
                nc.gpsimd.memset(wm, 0.0)
                for _ in range(WARMUP_GROUPS):
                    pw = ps.tile([128, 256], F32, tag="pw")
                    for k in range(8):
                        nc.tensor.matmul(
                            pw, wm[:, :128], wm[:, :],
                            start=(k == 0), stop=(k == 7),
                        )

            def mlp(x_tiles, chunk_list, y_sb, g_t, u_t, d_t, i_dim,
                    y_out_r, after_w0=None, w0_split=False, preload0=None,
                    at_hb=None):
                n_hb = i_dim // 512  # half-blocks of 512 intermediate cols
                for hb in range(n_hb):
                    g0_mblock = False
                    if hb == 0 and preload0 is not None:
                        g_sb, u_sb, d_sb = preload0
                    elif hb == 0 and w0_split:
                        # m-blocked layout: each [128, KH, 128] block is
                        # one contiguous run per partition; the first
                        # matmul only waits for block 0 + the first x.
                        # Interleave across the two fast hardware rings
                        # (sync/scalar ~200GB/s; the gpsimd ring is only
                        # ~100GB/s and starts late) in consumption order.
                        g0_mblock = True
                        g_sb = wp.tile([128, 4, KH, 128], BF16, tag="w")
                        u_sb = wp.tile([128, 4, KH, 128], BF16, tag="w")
                        for m in range(4):
                            eng = nc.sync if m % 2 == 0 else nc.scalar
                            eng.dma_start(out=g_sb[:, m], in_=gu_hb0_m(g_t, m))
                            eng.dma_start(out=u_sb[:, m], in_=gu_hb0_m(u_t, m))
                        d_sb = wp.tile([128, 4, H], BF16, tag="w")
                        nc.scalar.dma_start(out=d_sb, in_=d_hb(d_t, 0))
                    else:
                        g_sb = wp.tile([128, KH, 512], BF16, tag="w")
                        nc.sync.dma_start(out=g_sb, in_=gu_hb(g_t, hb))
                        u_sb = wp.tile([128, KH, 512], BF16, tag="w")
                        nc.sync.dma_start(out=u_sb, in_=gu_hb(u_t, hb))
                        d_sb = wp.tile([128, 4, H], BF16, tag="w")
                        nc.gpsimd.dma_start(out=d_sb, in_=d_hb(d_t, hb))

                    def g_sl(k, m):
                        if g0_mblock:
                            return g_sb[:, m, k, :]
                        return g_sb[:, k, m * 128 : (m + 1) * 128]

                    def u_sl(k, m):
                        if g0_mblock:
                            return u_sb[:, m, k, :]
                        return u_sb[:, k, m * 128 : (m + 1) * 128]

                    if hb == 0 and after_w0 is not None:
                        after_w0()
                    if at_hb is not None and hb in at_hb:
                        at_hb[hb]()
                    for ci, (c_off, cn) in enumerate(chunk_list):
                        x_sb = x_tiles[ci]
                        h_sb = hp.tile([128, 4, cn], BF16, tag="h")
                        x_sl = [x_sb[:, k, :] for k in range(KH)]
                        for m in range(4):
                            pg = ps.tile([128, cn], F32, tag="pg")
                            for k in range(KH):
                                nc.tensor.matmul(
                                    pg, g_sl(k, m), x_sl[k],
                                    start=(k == 0), stop=(k == KH - 1),
                                )
                            nc.scalar.activation(h_sb[:, m, :], pg, SILU)
                            pu = ps.tile([128, cn], F32, tag="pu")
                            for k in range(KH):
                                nc.tensor.matmul(
                                    pu, u_sl(k, m), x_sl[k],
                                    start=(k == 0), stop=(k == KH - 1),
                                )
                            nc.vector.tensor_mul(h_sb[:, m, :], h_sb[:, m, :], pu)
                        for hm in range(KH):
                            pd = ps.tile([128, cn], F32, tag="pd")
                            for k in range(4):
                                nc.tensor.matmul(
                                    pd,
                                    d_sb[:, k, hm * 128 : (hm + 1) * 128],
                                    h_sb[:, k, :],
                                    start=(k == 0), stop=(k == 3),
                                )
                            y_sl = y_sb[:, hm, c_off : c_off + cn]
                            if hb == 0:
                                nc.vector.tensor_copy(y_sl, pd)
                            elif hb < n_hb - 1:
                                nc.vector.tensor_add(y_sl, y_sl, pd)
                            else:
                                yo = op.tile([128, cn], BF16, tag="yo")
                                nc.vector.tensor_add(yo, y_sl, pd)
                                # never the scalar ring: a DMA trigger
                                # waiting for its data blocks the engine
                                # head-of-line, and scalar must keep
                                # running silu
                                eng = nc.sync if hm % 2 == 0 else nc.gpsimd
                                eng.dma_start(
                                    out=y_out_r[:, hm, c_off : c_off + cn],
                                    in_=yo,
                                )

            # ---- expert phase: C capacity tokens through this core's expert
            xe_tiles = [
                xp.tile([128, KH, cn], BF16, tag=f"xe{ci}", name=f"xe{ci}")
                for ci, (_, cn) in enumerate(chs_e)
            ]
            xs_tiles = [
                xp.tile([128, KH, cn], BF16, tag=f"xs{ci}", name=f"xs{ci}")
                for ci, (_, cn) in enumerate(chs_s)
            ]
            # first chunk's x: gates the first matmul
            nc.sync.dma_start(
                out=xe_tiles[0], in_=x_chunk(xT, 0, chs_e[0][1])
            )

            def after_w0():
                base = KH * chs_e[0][1]
                for ci in range(1, len(chs_e)):
                    cn = chs_e[ci][1]
                    nc.sync.dma_start(
                        out=xe_tiles[ci], in_=x_chunk(xT, base, cn)
                    )
                    base += KH * cn
                base = 0
                for ci, (_, cn) in enumerate(chs_s):
                    nc.sync.dma_start(
                        out=xs_tiles[ci], in_=x_chunk(xsT, base, cn)
                    )
                    base += KH * cn

            y_sb = yp.tile([128, KH, C], F32, tag="y")

            # shared-phase hb0 weights: dedicated tiles on the scalar
            # ring (idle after startup), prefetched with a priority that
            # slots them right after the startup DMAs — the wp pool's
            # rotating loads run just-in-time and made the phase
            # transition stall on these
            sw_g = swp.tile([128, KH, 512], BF16, tag="swg")
            sw_u = swp.tile([128, KH, 512], BF16, tag="swu")
            sw_d = swp.tile([128, 4, H], BF16, tag="swd")

            def prefetch_shared_w0():
                with tc.high_priority(offset=tc.cur_priority - 64):
                    nc.scalar.dma_start(out=sw_g, in_=gu_hb(sg, 0))
                    nc.scalar.dma_start(out=sw_u, in_=gu_hb(su, 0))
                    nc.scalar.dma_start(out=sw_d, in_=d_hb(sd, 0))

            mlp(xe_tiles, chs_e, y_sb, wg, wu, wd, I, yT_r,
                after_w0=after_w0, w0_split=True,
                at_hb={2: prefetch_shared_w0})

            # ---- shared-expert phase: this core's 1/8 shard of all tokens
            ys_sb = yp.tile([128, KH, NS], F32, tag="y")
            mlp(xs_tiles, chs_s, ys_sb, sg, su, sd, IS, ysT_r,
                preload0=(sw_g, sw_u, sw_d))

    nc.compile()
    return nc


def _pack_gu(w, m_block_hb0=False):
    """[K, N] gate/up weights -> [128, K//128 * N] per-partition-contiguous
    half-block-major blocks (hb0 m-blocked when requested)."""
    K, N = w.shape
    KT = K // 128
    w4 = w.reshape(KT, 128, N // 512, 512).transpose(1, 2, 0, 3)  # p hb k j
    if m_block_hb0:
        hb0 = w4[:, 0].reshape(128, KT, 4, 128).transpose(0, 2, 1, 3)
        return np.ascontiguousarray(
            np.concatenate(
                [hb0.reshape(128, -1), w4[:, 1:].reshape(128, -1)], axis=1
            )
        )
    return np.ascontiguousarray(w4.reshape(128, -1))


def _pack_d(w):
    """[I, H] down weights -> [128, I*H//128] half-block-major blocks."""
    I_, H_ = w.shape
    w4 = w.reshape(I_ // 512, 4, 128, H_).transpose(2, 0, 1, 3)  # p hb t j
    return np.ascontiguousarray(w4.reshape(128, -1))


def _pack_x(xTf, chunks):
    """[H, C] activations -> [128, H//128 * C] chunk-major blocks."""
    H_, C_ = xTf.shape
    xk = xTf.reshape(H_ // 128, 128, C_)
    return np.concatenate(
        [
            xk[:, :, lo : lo + sz].transpose(1, 0, 2).reshape(128, -1)
            for lo, sz in chunks
        ],
        axis=1,
    )


def _install_trace_hook():
    """run_bass_kernel_spmd(trace=True) under axon needs antenv.axon_hooks,
    absent from this image; shim it from trn_agent_boot."""
    import sys
    import types

    if "antenv.axon_hooks" in sys.modules:
        return
    from trn_agent_boot.trn_boot import _ntff_profile_via_ctypes

    hook = _ntff_profile_via_ctypes("/opt/axon/libaxon_pjrt.so")
    mod = types.ModuleType("antenv.axon_hooks")
    mod.get_axon_ntff_profile_hook = lambda: hook
    sys.modules["antenv.axon_hooks"] = mod


def kernel(hidden_states, Wr, Wg, Wu, Wd, Sg, Su, Sd):
    hidden_states = np.asarray(hidden_states, dtype=np.float32)
    Wr = np.asarray(Wr, dtype=np.float32)
    Wg = np.asarray(Wg, dtype=np.float32)
    Wu = np.asarray(Wu, dtype=np.float32)
    Wd = np.asarray(Wd, dtype=np.float32)
    Sg = np.asarray(Sg, dtype=np.float32)
    Su = np.asarray(Su, dtype=np.float32)
    Sd = np.asarray(Sd, dtype=np.float32)

    B, S, H = hidden_states.shape
    E = Wr.shape[1]
    I = Wg.shape[2]
    IS = Sg.shape[1]
    N = B * S
    assert E == N_CORES and N % N_CORES == 0
    NS = N // N_CORES

    flat = hidden_states.reshape(N, H)

    # host router, fp64 (softmax is monotone: top-k by logits == by probs)
    logits = flat.astype(np.float64) @ Wr.astype(np.float64)
    lm = logits.max(axis=1, keepdims=True)
    p = np.exp(logits - lm)
    p /= p.sum(axis=1, keepdims=True)
    order = np.argsort(-logits, axis=1, kind="stable")
    top = order[:, :TOP_K]

    sel = np.zeros((N, E), dtype=bool)
    np.put_along_axis(sel, top, True, axis=1)
    idx_e = [np.flatnonzero(sel[:, e]) for e in range(E)]
    counts = [len(ix) for ix in idx_e]
    C = max(512, math.ceil(max(counts) / 8) * 8)
    chs_e = _chunks(C, taper=True)
    chs_s = _chunks(NS)

    flatT = np.ascontiguousarray(flat.T.astype(NP_BF16))  # [H, N] bf16
    Sd_half = (Sd * np.float32(SHARED_SCALE)).astype(NP_BF16)
    sg_p = _pack_gu(Sg.astype(NP_BF16))
    su_p = _pack_gu(Su.astype(NP_BF16))
    sd_p = _pack_d(Sd_half)

    in_maps = []
    for e in range(E):
        ix = idx_e[e]
        cnt = counts[e]
        xT = np.zeros((H, C), NP_BF16)
        xT[:, :cnt] = flatT[:, ix]
        in_maps.append(
            {
                "xT": _pack_x(xT, chs_e),
                "wg": _pack_gu(Wg[e].astype(NP_BF16), m_block_hb0=True),
                "wu": _pack_gu(Wu[e].astype(NP_BF16), m_block_hb0=True),
                "wd": _pack_d(Wd[e].astype(NP_BF16)),
                "xsT": _pack_x(flatT[:, e * NS : (e + 1) * NS], chs_s),
                "sg": sg_p,
                "su": su_p,
                "sd": sd_p,
            }
        )

    key = (H, I, IS, C, NS)
    if key not in _NC_CACHE:
        _NC_CACHE[key] = _build(*key)
    nc = _NC_CACHE[key]

    run_kwargs = {}
    if TRACE:
        _install_trace_hook()
        import tempfile

        run_kwargs = {"trace": True, "tmpdir": tempfile.mkdtemp(prefix="moe_trace_")}
    res = run_bass_kernel_spmd(nc, in_maps, core_ids=list(range(N_CORES)), **run_kwargs)
    LAST["exec_time_ns"] = res.exec_time_ns
    LAST["profile_json"] = res.profile_json
    LAST["counts"] = counts
    LAST["C"] = C

    out = np.zeros((N, H), np.float32)
    for e in range(E):
        cnt = counts[e]
        ix = idx_e[e]
        w = p[ix, e].astype(np.float32)
        out[ix] += res.results[e]["yT"][:, :cnt].T.astype(np.float32) * w[:, None]
        out[e * NS : (e + 1) * NS] += res.results[e]["ysT"].T.astype(np.float32)
    return out.reshape(B, S, H)
